# revision 9
# baseline (speedup 1.0000x reference)
"""Trainium2 Bass kernel for nn_GruAgent — v2 (restructured recurrence).

Data-parallel over envs: 8 cores x 64 envs. Per core the GRU chain runs in
[H, B] layout. Critical-path redesign vs v1:

- State passed between steps as B = [a1; a2] stacked on 128 partitions with
  mh_t = a1 + a2 implicitly:  a1 = (mb*(1-z)) . n,  a2 = (mb*z) . mh_prev.
  The gate matmuls contract K=128 over duplicated weights [W;W], so the
  mask/blend multiplies move OFF the serial path (no mh materialisation on
  the path; the only post-tanh path op is a1 = n . mbu).
- One fused sigmoid for r|z on [128, 64].
- ginb (= gi_n + b_ih_n) precomputed per group into SBUF, so the n-gate arg
  is q = (ghn + b_hh_n) * r + ginb  ->  stt + add on DVE.
- Bulk matmuls (gi, heads, masks) run as float32r (1 cyc/row at N=512).
- Off-path elementwise split across Pool/DVE.
"""

import os
import sys

import numpy as np

for _p in ("/opt/trn_rl_repo", os.path.expanduser("~/.axon_site/_ro/trn_rl_repo")):
    if os.path.isdir(_p) and _p not in sys.path:
        sys.path.insert(0, _p)
        break

import concourse.bass as bass
import concourse.mybir as mybir
import concourse.tile as tile
from concourse import bacc
from concourse.masks import make_identity

T, B, OBS, H, A, L = 512, 512, 64, 64, 6, 64
N_CORES = 8
BL = B // N_CORES          # 64 envs per core
GS = 8                     # timesteps per group
COLS = GS * BL             # 512 columns per group
H3 = 3 * H

F32 = mybir.dt.float32
F32R = mybir.dt.float32r
AF = mybir.ActivationFunctionType
ALU = mybir.AluOpType

WEIGHT_KEYS = [
    "w_ih", "w_hh", "b_ih", "b_hh",
    "aw1", "ab1", "aw2", "ab2", "aw3", "ab3",
    "cw1", "cb1", "cw2", "cb2", "cw3", "cb3",
]


def r32(ap):
    return ap.bitcast(F32R)


def build(nc, t_loc=T):
    from contextlib import ExitStack

    assert t_loc % GS == 0
    ng = t_loc // GS

    x_d = nc.dram_tensor("x", [t_loc, BL, OBS], F32, kind="ExternalInput")
    done_d = nc.dram_tensor("done", [t_loc, BL], F32, kind="ExternalInput")
    h0_d = nc.dram_tensor("h0", [BL, H], F32, kind="ExternalInput")
    wih_d = nc.dram_tensor("w_ih", [H3, OBS], F32, kind="ExternalInput")
    whh_d = nc.dram_tensor("w_hh", [H3, H], F32, kind="ExternalInput")
    bih_d = nc.dram_tensor("b_ih", [H3], F32, kind="ExternalInput")
    bhh_d = nc.dram_tensor("b_hh", [H3], F32, kind="ExternalInput")
    aw1_d = nc.dram_tensor("aw1", [L, H + OBS], F32, kind="ExternalInput")
    ab1_d = nc.dram_tensor("ab1", [L], F32, kind="ExternalInput")
    aw2_d = nc.dram_tensor("aw2", [L, L], F32, kind="ExternalInput")
    ab2_d = nc.dram_tensor("ab2", [L], F32, kind="ExternalInput")
    aw3_d = nc.dram_tensor("aw3", [A, L], F32, kind="ExternalInput")
    ab3_d = nc.dram_tensor("ab3", [A], F32, kind="ExternalInput")
    cw1_d = nc.dram_tensor("cw1", [L, H + OBS], F32, kind="ExternalInput")
    cb1_d = nc.dram_tensor("cb1", [L], F32, kind="ExternalInput")
    cw2_d = nc.dram_tensor("cw2", [L, L], F32, kind="ExternalInput")
    cb2_d = nc.dram_tensor("cb2", [L], F32, kind="ExternalInput")
    cw3_d = nc.dram_tensor("cw3", [1, L], F32, kind="ExternalInput")
    cb3_d = nc.dram_tensor("cb3", [1], F32, kind="ExternalInput")
    out_d = nc.dram_tensor("out", [t_loc, BL, A + 1], F32, kind="ExternalOutput")

    with tile.TileContext(nc) as tc, ExitStack() as ctx:
        wp = ctx.enter_context(tc.tile_pool(name="wp", bufs=1))
        ldp = ctx.enter_context(tc.tile_pool(name="ldp", bufs=2))
        catp = ctx.enter_context(tc.tile_pool(name="catp", bufs=4))
        xnp = ctx.enter_context(tc.tile_pool(name="xnp", bufs=2))
        drp = ctx.enter_context(tc.tile_pool(name="drp", bufs=2))
        mbp = ctx.enter_context(tc.tile_pool(name="mbp", bufs=3))
        gbp = ctx.enter_context(tc.tile_pool(name="gbp", bufs=2))
        small = ctx.enter_context(tc.tile_pool(name="small", bufs=4))
        tmlp = ctx.enter_context(tc.tile_pool(name="tmlp", bufs=3))
        onp = ctx.enter_context(tc.tile_pool(name="onp", bufs=2))

        przEp = ctx.enter_context(tc.tile_pool(name="przEp", bufs=2, space="PSUM"))
        przOp = ctx.enter_context(tc.tile_pool(name="przOp", bufs=2, space="PSUM"))
        pghnp = ctx.enter_context(tc.tile_pool(name="pghnp", bufs=2, space="PSUM"))
        pmisc = ctx.enter_context(tc.tile_pool(name="pmisc", bufs=2, space="PSUM"))

        CH = 512  # bulk op chunk width (bounds engine-queuing ahead of path ops)

        def chunked(fn, cols=COLS, ch=CH):
            for c0 in range(0, cols, ch):
                fn(slice(c0, min(c0 + ch, cols)))

        ident = wp.tile([128, 128], F32, tag="ident")
        make_identity(nc, ident[:])
        zscratch = wp.tile([128, 128], F32, tag="zscratch")
        nc.vector.memset(zscratch[:], 0.0)

        def load_transposed(dram_ap, rows, cols, tag, dt=F32):
            """dram [rows, cols] -> sbuf [cols, rows]."""
            dst = wp.tile([cols, rows], dt, tag=tag)
            r0 = 0
            while r0 < rows:
                rr = min(128, rows - r0)
                tmp = ldp.tile([128, 128], F32, tag="wtmp")
                nc.sync.dma_start(tmp[:rr, :cols], dram_ap[r0:r0 + rr, :])
                pt = pmisc.tile([128, COLS], F32, tag="pm")
                nc.tensor.transpose(pt[:cols, :rr], tmp[:rr, :cols], ident[:rr, :rr])
                nc.scalar.copy(dst[:cols, r0:r0 + rr], pt[:cols, :rr])
                r0 += rr
            return dst

        def load_col(dram_1d, n, tag, off=0, dst=None, dst_off=0):
            if dst is None:
                dst = wp.tile([max(n + dst_off, 1), 1], F32, tag=tag)
            nc.sync.dma_start(
                dst[dst_off:dst_off + n, :],
                dram_1d[off:off + n].rearrange("p -> p ()"),
            )
            return dst

        # --- weights / constants (once, overlapped with first group) ---
        w_ihT = load_transposed(wih_d[:], H3, OBS, "wihT", dt=F32R)        # [64, 192]
        whhT = load_transposed(whh_d[:], H3, H, "whhT", dt=F32R)            # [64, 192]
        wrz = whhT[:, 0:128]                                       # lhsT [64,128]
        wn = whhT[:, 128:H3]                                       # lhsT [64,64]
        h0T = load_transposed(h0_d[:], BL, H, "h0T", dt=F32R)               # [64, 64]

        lhsT1h = wp.tile([64, 128], F32R, tag="lhsT1h")
        lhsT1x = wp.tile([64, 128], F32R, tag="lhsT1x")
        for src, c0 in ((aw1_d, 0), (cw1_d, 64)):
            tmp = ldp.tile([128, 128], F32, tag="wtmp")
            nc.sync.dma_start(tmp[:L, :H + OBS], src[:, :])
            pt = pmisc.tile([128, COLS], F32, tag="pm")
            nc.tensor.transpose(pt[:H, :L], tmp[:L, 0:H], ident[:L, :L])
            nc.tensor.transpose(pt[:OBS, 128:128 + L], tmp[:L, H:H + OBS], ident[:L, :L])
            nc.scalar.copy(lhsT1h[:, c0:c0 + L], pt[:H, :L])
            nc.scalar.copy(lhsT1x[:, c0:c0 + L], pt[:OBS, 128:128 + L])

        lhsT2 = wp.tile([128, 128], F32R, tag="lhsT2")
        nc.scalar.copy(lhsT2[:], zscratch[:])
        for src, o in ((aw2_d, 0), (cw2_d, 64)):
            tmp = ldp.tile([128, 128], F32, tag="wtmp")
            nc.sync.dma_start(tmp[:L, :L], src[:, :])
            pt = pmisc.tile([128, COLS], F32, tag="pm")
            nc.tensor.transpose(pt[:L, :L], tmp[:L, :L], ident[:L, :L])
            nc.scalar.copy(lhsT2[o:o + L, o:o + L], pt[:L, :L])

        lhsT3 = wp.tile([128, A + 1], F32R, tag="lhsT3")
        nc.scalar.copy(lhsT3[:], zscratch[:, 0:A + 1])
        tmp = ldp.tile([128, 128], F32, tag="wtmp")
        nc.sync.dma_start(tmp[:A, :L], aw3_d[:, :])
        pt = pmisc.tile([128, COLS], F32, tag="pm")
        nc.tensor.transpose(pt[:L, :A], tmp[:A, :L], ident[:A, :A])
        nc.scalar.copy(lhsT3[:L, :A], pt[:L, :A])
        tmp = ldp.tile([128, 128], F32, tag="wtmp")
        nc.sync.dma_start(tmp[:1, :L], cw3_d[:, :])
        pt = pmisc.tile([128, COLS], F32, tag="pm")
        nc.tensor.transpose(pt[:L, :1], tmp[:1, :L], ident[:1, :1])
        nc.scalar.copy(lhsT3[64:64 + L, A:A + 1], pt[:L, :1])

        # biases
        bihc = load_col(bih_d, 64, "bihc")
        bhhc = load_col(bhh_d, 64, "bhhc")
        bias_r = wp.tile([64, 1], F32, tag="bias_r")
        nc.vector.tensor_add(bias_r[:], bihc[:], bhhc[:])
        bihz = load_col(bih_d, 64, "bihz", off=64)
        bhhz = load_col(bhh_d, 64, "bhhz", off=64)
        bias_z = wp.tile([64, 1], F32, tag="bias_z")
        nc.vector.tensor_add(bias_z[:], bihz[:], bhhz[:])
        b_ihn = load_col(bih_d, H, "b_ihn", off=128)               # [64,1]
        b_hhn = load_col(bhh_d, H, "b_hhn", off=128)               # [64,1]

        bias1 = wp.tile([128, 1], F32, tag="bias1")
        load_col(ab1_d, L, "bias1", dst=bias1, dst_off=0)
        load_col(cb1_d, L, "bias1", dst=bias1, dst_off=64)
        bias2 = wp.tile([128, 1], F32, tag="bias2")
        load_col(ab2_d, L, "bias2", dst=bias2, dst_off=0)
        load_col(cb2_d, L, "bias2", dst=bias2, dst_off=64)
        bias3 = wp.tile([A + 1, 1], F32, tag="bias3")
        load_col(ab3_d, A, "bias3", dst=bias3, dst_off=0)
        load_col(cb3_d, 1, "bias3", dst=bias3, dst_off=A)

        ones_row = wp.tile([1, BL], F32, tag="ones_row")
        nc.vector.memset(ones_row[:], 1.0)

        # --- group bulk: x load/transpose, masks, gi projections ---
        def bulk(g):
            vg = catp.tile([64, COLS], F32R, tag="vg")
            zg = catp.tile([64, COLS], F32R, tag="zg")
            xT = catp.tile([64, COLS], F32R, tag="xT")
            xn = xnp.tile([128, GS // 2, OBS], F32, tag="xn")
            nc.sync.dma_start(
                xn[:],
                x_d[g * GS:(g + 1) * GS].rearrange("(k ph) b f -> (ph b) k f", ph=2),
            )
            ptx = pmisc.tile([128, COLS], F32, tag="pm")
            for k in range(GS // 2):
                nc.tensor.transpose(
                    ptx[:OBS, k * 128:(k + 1) * 128], xn[:, k, :], ident[:, :]
                )
            chunked(lambda sl: nc.vector.tensor_copy(xT[:, sl], ptx[:OBS, sl]))

            dr = drp.tile([1, COLS], F32, tag="dr")
            nc.sync.dma_start(
                dr[:], done_d[g * GS:(g + 1) * GS].rearrange("t b -> () (t b)")
            )
            pmb = pmisc.tile([128, COLS], F32, tag="pm")
            nc.tensor.matmul(pmb[:BL, :], ones_row[:], dr[:],
                             start=True, stop=True)
            mb = mbp.tile([BL, COLS], F32R, tag="mb")
            chunked(lambda sl: nc.scalar.activation(
                mb[:, sl], pmb[:BL, sl], AF.Identity, scale=-1.0, bias=1.0))

            xT4 = xT[:].rearrange("p (k q b) -> p k q b", k=GS // 2, q=2)
            przE = przEp.tile([128, COLS // 2], F32, tag="przE")
            przO = przOp.tile([128, COLS // 2], F32, tag="przO")
            nc.tensor.matmul(
                przE[:], w_ihT[:, 0:128], xT4[:, :, 0, :],
                start=True, stop=False, skip_group_check=True,
            )
            nc.tensor.matmul(
                przO[:], w_ihT[:, 0:128], xT4[:, :, 1, :],
                start=True, stop=False, skip_group_check=True,
            )
            pgn = pmisc.tile([128, COLS], F32, tag="pm")
            chunked(lambda sl: nc.tensor.matmul(
                pgn[:BL, sl], w_ihT[:, 128:H3], xT[:, sl],
                start=True, stop=True, skip_group_check=True,
            ))
            ginb = gbp.tile([64, COLS], F32, tag="ginb")
            chunked(lambda sl: nc.scalar.activation(
                ginb[:, sl], pgn[:BL, sl], AF.Identity, bias=b_ihn[:]))
            return dict(vg=vg, zg=zg, xT=xT, mb=mb, przE=przE, przO=przO, ginb=ginb)

        state = {}

        def chain(g, refs, refs_next):
            ginb, mb = refs["ginb"], refs["mb"]
            vg, zg = refs["vg"], refs["zg"]
            for s in range(GS):
                t = g * GS + s
                # even/odd steps accumulate in separate PSUM banks so step
                # s+1's matmuls never serialize behind step s's sigmoid read
                # (Tile's PSUM bank-collision guard).
                prz = refs["przE"] if s % 2 == 0 else refs["przO"]
                cs = bass.ts(s, BL)          # group-layout columns (ginb/zg/vg)
                cp = bass.ts(s // 2, BL)     # parity-bank columns (prz)
                a1 = state["a1"]
                mh = state["mh"]
                # a2's matmuls were already emitted last step (right after a2
                # was produced) so their sem-waits don't couple to a1; only
                # the a1 pair is emitted here, on the critical path.
                pghn = state["pghn"]
                nc.tensor.matmul(
                    prz[:, cp], wrz, a1[:],
                    start=False, stop=(s >= GS - 2), skip_group_check=True,
                )
                nc.tensor.matmul(pghn[:], wn, a1[:], start=False, stop=True,
                                 skip_group_check=True)
                r_t = small.tile([BL, BL], F32, tag="r_t")
                nc.scalar.activation(r_t[:], prz[0:64, cp], AF.Sigmoid, bias=bias_r[:])
                z_t = small.tile([BL, BL], F32R, tag="z_t")
                nc.scalar.activation(z_t[:], prz[64:128, cp], AF.Sigmoid, bias=bias_z[:])
                # critical path on DVE only: p -> q -> (tanh) -> a1
                p = small.tile([BL, BL], F32, tag="p")
                nc.vector.scalar_tensor_tensor(
                    p[:], pghn[:], b_hhn[:], r_t[:], ALU.add, ALU.mult
                )
                q = small.tile([BL, BL], F32, tag="q")
                nc.vector.tensor_add(q[:], p[:], ginb[:, cs])
                n = small.tile([BL, BL], F32R, tag="n")
                nc.scalar.activation(n[:], q[:], AF.Tanh)

                # off-path (Pool): u, z*mh, u*n, masks for next step
                if t < t_loc - 1:
                    if s == GS - 1:
                        mbn = refs_next["mb"][:, 0:BL]
                    else:
                        mbn = mb[:, bass.ts(s + 1, BL)]
                u = small.tile([BL, BL], F32R, tag="u")
                nc.gpsimd.tensor_scalar(u[:], z_t[:], -1.0, 1.0,
                                        ALU.mult, ALU.add)
                nc.gpsimd.tensor_mul(zg[:, cs], z_t[:], mh[:])

                if t < t_loc - 1:
                    # Pool queue is in-order: emit everything feeding the path
                    # op a1 = n*mbu before the n-dependent vg, so mbu is never
                    # stuck behind it.
                    mbu = small.tile([BL, BL], F32R, tag="mbu")
                    nc.gpsimd.tensor_mul(mbu[:], u[:], mbn)
                    a2n = small.tile([BL, BL], F32R, tag="a2")
                    nc.gpsimd.tensor_mul(a2n[:], zg[:, cs], mbn)
                    a1n = small.tile([BL, BL], F32R, tag="a1")
                    nc.vector.tensor_mul(a1n[:], n[:], mbu[:])          # path
                nc.gpsimd.tensor_mul(vg[:, cs], u[:], n[:])
                if t < t_loc - 1:
                    mh2 = small.tile([BL, BL], F32R, tag="mh")
                    nc.gpsimd.tensor_add(mh2[:], a1n[:], a2n[:])
                    # emit next step's a2 matmuls NOW (decoupled from a1)
                    if s == GS - 1:
                        prz_nx = refs_next["przE"]
                        cs_nx = bass.ts(0, BL)
                    else:
                        prz_nx = refs["przE"] if (s + 1) % 2 == 0 else refs["przO"]
                        cs_nx = bass.ts((s + 1) // 2, BL)
                    pghn_nx = pghnp.tile([64, BL], F32, tag="pghn")
                    nc.tensor.matmul(
                        prz_nx[:, cs_nx], wrz, a2n[:],
                        start=False, stop=False, skip_group_check=True,
                    )
                    nc.tensor.matmul(pghn_nx[:], wn, a2n[:], start=True,
                                     stop=False, skip_group_check=True)
                    state["a1"], state["mh"] = a1n, mh2
                    state["pghn"] = pghn_nx

        def head(g, refs):
            vg, zg, xT = refs["vg"], refs["zg"], refs["xT"]
            p1 = pmisc.tile([128, COLS], F32, tag="pm")
            chunked(lambda sl: nc.tensor.matmul(
                p1[:, sl], lhsT1h[:], vg[:, sl], start=True, stop=False,
                skip_group_check=True))
            chunked(lambda sl: nc.tensor.matmul(
                p1[:, sl], lhsT1h[:], zg[:, sl], start=False, stop=False,
                skip_group_check=True))
            chunked(lambda sl: nc.tensor.matmul(
                p1[:, sl], lhsT1x[:], xT[:, sl], start=False, stop=True,
                skip_group_check=True))
            t1 = tmlp.tile([128, COLS], F32R, tag="t1")
            chunked(lambda sl: nc.scalar.activation(
                t1[:, sl], p1[:, sl], AF.Tanh, bias=bias1[:]))
            p2 = pmisc.tile([128, COLS], F32, tag="pm")
            chunked(lambda sl: nc.tensor.matmul(
                p2[:, sl], lhsT2[:], t1[:, sl], start=True, stop=True,
                skip_group_check=True))
            t2 = tmlp.tile([128, COLS], F32R, tag="t2")
            chunked(lambda sl: nc.scalar.activation(
                t2[:, sl], p2[:, sl], AF.Tanh, bias=bias2[:]))
            p3 = pmisc.tile([128, COLS], F32, tag="pm")
            chunked(lambda sl: nc.tensor.matmul(
                p3[:A + 1, sl], lhsT3[:], t2[:, sl],
                start=True, stop=True, skip_group_check=True))
            o7 = tmlp.tile([A + 1, COLS], F32, tag="o7")
            chunked(lambda sl: nc.vector.tensor_scalar_add(
                o7[:, sl], p3[:A + 1, sl], bias3[:]))

            po = pmisc.tile([128, GS // 2, A + 1], F32, tag="pm")
            for k in range(GS // 2):
                nc.tensor.transpose(
                    po[:, k, :], o7[:, k * 128:(k + 1) * 128], ident[:A + 1, :A + 1]
                )
            on = onp.tile([128, GS // 2, A + 1], F32, tag="on")
            nc.vector.tensor_copy(on[:], po[:])
            nc.sync.dma_start(
                out_d[g * GS:(g + 1) * GS].rearrange("(k ph) b j -> (ph b) k j", ph=2),
                on[:],
            )

        refs = bulk(0)
        # initial state: a1 = mb0 * h0T, a2 = 0, mh = a1
        a1_0 = small.tile([BL, BL], F32R, tag="a1")
        nc.vector.tensor_mul(a1_0[:], h0T[:], refs["mb"][:, 0:BL])
        a2_0 = small.tile([BL, BL], F32R, tag="a2")
        nc.scalar.copy(a2_0[:], zscratch[0:BL, 0:BL])
        pghn_0 = pghnp.tile([64, BL], F32, tag="pghn")
        nc.tensor.matmul(
            refs["przE"][:, bass.ts(0, BL)], wrz, a2_0[:],
            start=False, stop=False, skip_group_check=True,
        )
        nc.tensor.matmul(pghn_0[:], wn, a2_0[:], start=True, stop=False,
                         skip_group_check=True)
        state["a1"], state["mh"] = a1_0, a1_0
        state["pghn"] = pghn_0
        for g in range(1, ng):
            refs_next = bulk(g)
            chain(g - 1, refs, refs_next)
            head(g - 1, refs)
            refs = refs_next
        chain(ng - 1, refs, None)
        head(ng - 1, refs)

    return nc


_BUILT = {}


def get_built(t_loc=T):
    if t_loc not in _BUILT:
        nc = bacc.Bacc(None, target_bir_lowering=False)
        build(nc, t_loc)
        nc.compile()
        _BUILT[t_loc] = nc
    return _BUILT[t_loc]


def shard_inputs(inputs, t_loc=T):
    x = np.ascontiguousarray(np.asarray(inputs["x"], np.float32)).reshape(t_loc, B, OBS)
    done = np.ascontiguousarray(np.asarray(inputs["done"], np.float32)).reshape(t_loc, B)
    h0 = np.ascontiguousarray(np.asarray(inputs["gru_state"], np.float32)).reshape(B, H)
    common = {
        k: np.ascontiguousarray(np.asarray(inputs[k], np.float32))
        for k in WEIGHT_KEYS
    }
    in_maps = []
    for c in range(N_CORES):
        sl = slice(c * BL, (c + 1) * BL)
        m = dict(common)
        m["x"] = np.ascontiguousarray(x[:, sl, :])
        m["done"] = np.ascontiguousarray(done[:, sl])
        m["h0"] = np.ascontiguousarray(h0[sl, :])
        in_maps.append(m)
    return in_maps


def assemble_output(per_core_outs, t_loc=T):
    outs = [np.asarray(o, np.float32).reshape(t_loc, BL, A + 1) for o in per_core_outs]
    full = np.stack(outs, axis=1).reshape(t_loc, B, A + 1)
    return np.ascontiguousarray(full.reshape(t_loc * B, A + 1))


def run_on_hw(inputs, t_loc=T, trace=False, **kw):
    from concourse.bass_utils import run_bass_kernel_spmd

    nc = get_built(t_loc)
    in_maps = shard_inputs(inputs, t_loc)
    res = run_bass_kernel_spmd(
        nc, in_maps, core_ids=list(range(N_CORES)), trace=trace, **kw
    )
    out = assemble_output([r["out"] for r in res.results], t_loc)
    return out, res


def kernel(**inputs):
    out, _ = run_on_hw(inputs)
    return out


# revision 10
# speedup vs baseline: 1.1512x; 1.1512x over previous
"""Trainium2 Bass kernel for nn_GruAgent — v2 (restructured recurrence).

Data-parallel over envs: 8 cores x 64 envs. Per core the GRU chain runs in
[H, B] layout. Critical-path redesign vs v1:

- State passed between steps as B = [a1; a2] stacked on 128 partitions with
  mh_t = a1 + a2 implicitly:  a1 = (mb*(1-z)) . n,  a2 = (mb*z) . mh_prev.
  The gate matmuls contract K=128 over duplicated weights [W;W], so the
  mask/blend multiplies move OFF the serial path (no mh materialisation on
  the path; the only post-tanh path op is a1 = n . mbu).
- One fused sigmoid for r|z on [128, 64].
- ginb (= gi_n + b_ih_n) precomputed per group into SBUF, so the n-gate arg
  is q = (ghn + b_hh_n) * r + ginb  ->  stt + add on DVE.
- Bulk matmuls (gi, heads, masks) run as float32r (1 cyc/row at N=512).
- Off-path elementwise split across Pool/DVE.
"""

import os
import sys

import numpy as np

for _p in ("/opt/trn_rl_repo", os.path.expanduser("~/.axon_site/_ro/trn_rl_repo")):
    if os.path.isdir(_p) and _p not in sys.path:
        sys.path.insert(0, _p)
        break

import concourse.bass as bass
import concourse.mybir as mybir
import concourse.tile as tile
from concourse import bacc
from concourse.masks import make_identity

T, B, OBS, H, A, L = 512, 512, 64, 64, 6, 64
N_CORES = 8
BL = B // N_CORES          # 64 envs per core
GS = 8                     # timesteps per group
COLS = GS * BL             # 512 columns per group
H3 = 3 * H

F32 = mybir.dt.float32
F32R = mybir.dt.float32r
AF = mybir.ActivationFunctionType
ALU = mybir.AluOpType

WEIGHT_KEYS = [
    "w_ih", "w_hh", "b_ih", "b_hh",
    "aw1", "ab1", "aw2", "ab2", "aw3", "ab3",
    "cw1", "cb1", "cw2", "cb2", "cw3", "cb3",
]


def r32(ap):
    return ap.bitcast(F32R)


def build(nc, t_loc=T):
    from contextlib import ExitStack

    assert t_loc % GS == 0
    ng = t_loc // GS

    x_d = nc.dram_tensor("x", [t_loc, BL, OBS], F32, kind="ExternalInput")
    done_d = nc.dram_tensor("done", [t_loc, BL], F32, kind="ExternalInput")
    h0_d = nc.dram_tensor("h0", [BL, H], F32, kind="ExternalInput")
    wih_d = nc.dram_tensor("w_ih", [H3, OBS], F32, kind="ExternalInput")
    whh_d = nc.dram_tensor("w_hh", [H3, H], F32, kind="ExternalInput")
    bih_d = nc.dram_tensor("b_ih", [H3], F32, kind="ExternalInput")
    bhh_d = nc.dram_tensor("b_hh", [H3], F32, kind="ExternalInput")
    aw1_d = nc.dram_tensor("aw1", [L, H + OBS], F32, kind="ExternalInput")
    ab1_d = nc.dram_tensor("ab1", [L], F32, kind="ExternalInput")
    aw2_d = nc.dram_tensor("aw2", [L, L], F32, kind="ExternalInput")
    ab2_d = nc.dram_tensor("ab2", [L], F32, kind="ExternalInput")
    aw3_d = nc.dram_tensor("aw3", [A, L], F32, kind="ExternalInput")
    ab3_d = nc.dram_tensor("ab3", [A], F32, kind="ExternalInput")
    cw1_d = nc.dram_tensor("cw1", [L, H + OBS], F32, kind="ExternalInput")
    cb1_d = nc.dram_tensor("cb1", [L], F32, kind="ExternalInput")
    cw2_d = nc.dram_tensor("cw2", [L, L], F32, kind="ExternalInput")
    cb2_d = nc.dram_tensor("cb2", [L], F32, kind="ExternalInput")
    cw3_d = nc.dram_tensor("cw3", [1, L], F32, kind="ExternalInput")
    cb3_d = nc.dram_tensor("cb3", [1], F32, kind="ExternalInput")
    out_d = nc.dram_tensor("out", [t_loc, BL, A + 1], F32, kind="ExternalOutput")

    with tile.TileContext(nc) as tc, ExitStack() as ctx:
        wp = ctx.enter_context(tc.tile_pool(name="wp", bufs=1))
        ldp = ctx.enter_context(tc.tile_pool(name="ldp", bufs=2))
        catp = ctx.enter_context(tc.tile_pool(name="catp", bufs=4))
        xnp = ctx.enter_context(tc.tile_pool(name="xnp", bufs=2))
        drp = ctx.enter_context(tc.tile_pool(name="drp", bufs=2))
        mbp = ctx.enter_context(tc.tile_pool(name="mbp", bufs=3))
        gbp = ctx.enter_context(tc.tile_pool(name="gbp", bufs=2))
        small = ctx.enter_context(tc.tile_pool(name="small", bufs=4))
        tmlp = ctx.enter_context(tc.tile_pool(name="tmlp", bufs=3))
        onp = ctx.enter_context(tc.tile_pool(name="onp", bufs=2))

        przEp = ctx.enter_context(tc.tile_pool(name="przEp", bufs=2, space="PSUM"))
        przOp = ctx.enter_context(tc.tile_pool(name="przOp", bufs=2, space="PSUM"))
        pghnp = ctx.enter_context(tc.tile_pool(name="pghnp", bufs=2, space="PSUM"))
        pmisc = ctx.enter_context(tc.tile_pool(name="pmisc", bufs=2, space="PSUM"))

        CH = 512  # bulk op chunk width (bounds engine-queuing ahead of path ops)

        def chunked(fn, cols=COLS, ch=CH):
            for c0 in range(0, cols, ch):
                fn(slice(c0, min(c0 + ch, cols)))

        ident = wp.tile([128, 128], F32, tag="ident")
        make_identity(nc, ident[:])
        zscratch = wp.tile([128, 128], F32, tag="zscratch")
        nc.vector.memset(zscratch[:], 0.0)

        def load_transposed(dram_ap, rows, cols, tag, dt=F32):
            """dram [rows, cols] -> sbuf [cols, rows]."""
            dst = wp.tile([cols, rows], dt, tag=tag)
            r0 = 0
            while r0 < rows:
                rr = min(128, rows - r0)
                tmp = ldp.tile([128, 128], F32, tag="wtmp")
                nc.sync.dma_start(tmp[:rr, :cols], dram_ap[r0:r0 + rr, :])
                pt = pmisc.tile([128, COLS], F32, tag="pm")
                nc.tensor.transpose(pt[:cols, :rr], tmp[:rr, :cols], ident[:rr, :rr])
                nc.scalar.copy(dst[:cols, r0:r0 + rr], pt[:cols, :rr])
                r0 += rr
            return dst

        def load_col(dram_1d, n, tag, off=0, dst=None, dst_off=0):
            if dst is None:
                dst = wp.tile([max(n + dst_off, 1), 1], F32, tag=tag)
            nc.sync.dma_start(
                dst[dst_off:dst_off + n, :],
                dram_1d[off:off + n].rearrange("p -> p ()"),
            )
            return dst

        # --- weights / constants (once, overlapped with first group) ---
        w_ihT = load_transposed(wih_d[:], H3, OBS, "wihT", dt=F32R)        # [64, 192]
        whhT = load_transposed(whh_d[:], H3, H, "whhT", dt=F32R)            # [64, 192]
        wrz = whhT[:, 0:128]                                       # lhsT [64,128]
        wn = whhT[:, 128:H3]                                       # lhsT [64,64]
        h0T = load_transposed(h0_d[:], BL, H, "h0T", dt=F32R)               # [64, 64]

        # biases
        bihc = load_col(bih_d, 64, "bihc")
        bhhc = load_col(bhh_d, 64, "bhhc")
        bias_r = wp.tile([64, 1], F32, tag="bias_r")
        nc.vector.tensor_add(bias_r[:], bihc[:], bhhc[:])
        bihz = load_col(bih_d, 64, "bihz", off=64)
        bhhz = load_col(bhh_d, 64, "bhhz", off=64)
        bias_z = wp.tile([64, 1], F32, tag="bias_z")
        nc.vector.tensor_add(bias_z[:], bihz[:], bhhz[:])
        b_ihn = load_col(bih_d, H, "b_ihn", off=128)               # [64,1]
        b_hhn = load_col(bhh_d, H, "b_hhn", off=128)               # [64,1]

        bias1 = wp.tile([128, 1], F32, tag="bias1")
        load_col(ab1_d, L, "bias1", dst=bias1, dst_off=0)
        load_col(cb1_d, L, "bias1", dst=bias1, dst_off=64)
        bias2 = wp.tile([128, 1], F32, tag="bias2")
        load_col(ab2_d, L, "bias2", dst=bias2, dst_off=0)
        load_col(cb2_d, L, "bias2", dst=bias2, dst_off=64)
        bias3 = wp.tile([A + 1, 1], F32, tag="bias3")
        load_col(ab3_d, A, "bias3", dst=bias3, dst_off=0)
        load_col(cb3_d, 1, "bias3", dst=bias3, dst_off=A)

        ones_row = wp.tile([1, BL], F32, tag="ones_row")
        nc.vector.memset(ones_row[:], 1.0)

        # --- group bulk: x load/transpose, masks, gi projections ---
        def bulk(g):
            vg = catp.tile([64, COLS], F32R, tag="vg")
            zg = catp.tile([64, COLS], F32R, tag="zg")
            xT = catp.tile([64, COLS], F32R, tag="xT")
            xn = xnp.tile([128, GS // 2, OBS], F32, tag="xn")
            nc.sync.dma_start(
                xn[:],
                x_d[g * GS:(g + 1) * GS].rearrange("(k ph) b f -> (ph b) k f", ph=2),
            )
            ptx = pmisc.tile([128, COLS], F32, tag="pm")
            for k in range(GS // 2):
                nc.tensor.transpose(
                    ptx[:OBS, k * 128:(k + 1) * 128], xn[:, k, :], ident[:, :]
                )
            chunked(lambda sl: nc.vector.tensor_copy(xT[:, sl], ptx[:OBS, sl]))

            dr = drp.tile([1, COLS], F32, tag="dr")
            nc.sync.dma_start(
                dr[:], done_d[g * GS:(g + 1) * GS].rearrange("t b -> () (t b)")
            )
            pmb = pmisc.tile([128, COLS], F32, tag="pm")
            nc.tensor.matmul(pmb[:BL, :], ones_row[:], dr[:],
                             start=True, stop=True)
            mb = mbp.tile([BL, COLS], F32R, tag="mb")
            chunked(lambda sl: nc.scalar.activation(
                mb[:, sl], pmb[:BL, sl], AF.Identity, scale=-1.0, bias=1.0))

            xT4 = xT[:].rearrange("p (k q b) -> p k q b", k=GS // 2, q=2)
            przE = przEp.tile([128, COLS // 2], F32, tag="przE")
            przO = przOp.tile([128, COLS // 2], F32, tag="przO")
            nc.tensor.matmul(
                przE[:], w_ihT[:, 0:128], xT4[:, :, 0, :],
                start=True, stop=False, skip_group_check=True,
            )
            nc.tensor.matmul(
                przO[:], w_ihT[:, 0:128], xT4[:, :, 1, :],
                start=True, stop=False, skip_group_check=True,
            )
            pgn = pmisc.tile([128, COLS], F32, tag="pm")
            chunked(lambda sl: nc.tensor.matmul(
                pgn[:BL, sl], w_ihT[:, 128:H3], xT[:, sl],
                start=True, stop=True, skip_group_check=True,
            ))
            ginb = gbp.tile([64, COLS], F32, tag="ginb")
            chunked(lambda sl: nc.scalar.activation(
                ginb[:, sl], pgn[:BL, sl], AF.Identity, bias=b_ihn[:]))
            return dict(vg=vg, zg=zg, xT=xT, mb=mb, przE=przE, przO=przO, ginb=ginb)

        state = {}

        def chain(g, refs, refs_next):
            ginb, mb = refs["ginb"], refs["mb"]
            vg, zg = refs["vg"], refs["zg"]
            for s in range(GS):
                t = g * GS + s
                # even/odd steps accumulate in separate PSUM banks so step
                # s+1's matmuls never serialize behind step s's sigmoid read
                # (Tile's PSUM bank-collision guard).
                prz = refs["przE"] if s % 2 == 0 else refs["przO"]
                cs = bass.ts(s, BL)          # group-layout columns (ginb/zg/vg)
                cp = bass.ts(s // 2, BL)     # parity-bank columns (prz)
                a1 = state["a1"]
                mh = state["mh"]
                # a2's matmuls were already emitted last step (right after a2
                # was produced) so their sem-waits don't couple to a1; only
                # the a1 pair is emitted here, on the critical path.
                pghn = state["pghn"]
                nc.tensor.matmul(
                    prz[:, cp], wrz, a1[:],
                    start=False, stop=(s >= GS - 2), skip_group_check=True,
                )
                nc.tensor.matmul(pghn[:], wn, a1[:], start=False, stop=True,
                                 skip_group_check=True)
                r_t = small.tile([BL, BL], F32, tag="r_t")
                nc.scalar.activation(r_t[:], prz[0:64, cp], AF.Sigmoid, bias=bias_r[:])
                z_t = small.tile([BL, BL], F32R, tag="z_t")
                nc.scalar.activation(z_t[:], prz[64:128, cp], AF.Sigmoid, bias=bias_z[:])
                # critical path on DVE only: p -> q -> (tanh) -> a1
                p = small.tile([BL, BL], F32, tag="p")
                nc.vector.scalar_tensor_tensor(
                    p[:], pghn[:], b_hhn[:], r_t[:], ALU.add, ALU.mult
                )
                q = small.tile([BL, BL], F32, tag="q")
                nc.vector.tensor_add(q[:], p[:], ginb[:, cs])
                n = small.tile([BL, BL], F32R, tag="n")
                nc.scalar.activation(n[:], q[:], AF.Tanh)

                # off-path (Pool): u, z*mh, u*n, masks for next step
                if t < t_loc - 1:
                    if s == GS - 1:
                        mbn = refs_next["mb"][:, 0:BL]
                    else:
                        mbn = mb[:, bass.ts(s + 1, BL)]
                u = small.tile([BL, BL], F32R, tag="u")
                nc.gpsimd.tensor_scalar(u[:], z_t[:], -1.0, 1.0,
                                        ALU.mult, ALU.add)
                nc.gpsimd.tensor_mul(zg[:, cs], z_t[:], mh[:])

                if t < t_loc - 1:
                    # Pool queue is in-order: emit everything feeding the path
                    # op a1 = n*mbu before the n-dependent vg, so mbu is never
                    # stuck behind it.
                    mbu = small.tile([BL, BL], F32R, tag="mbu")
                    nc.gpsimd.tensor_mul(mbu[:], u[:], mbn)
                    a2n = small.tile([BL, BL], F32R, tag="a2")
                    nc.gpsimd.tensor_mul(a2n[:], zg[:, cs], mbn)
                    a1n = small.tile([BL, BL], F32R, tag="a1")
                    nc.vector.tensor_mul(a1n[:], n[:], mbu[:])          # path
                nc.gpsimd.tensor_mul(vg[:, cs], u[:], n[:])
                if t < t_loc - 1:
                    mh2 = small.tile([BL, BL], F32R, tag="mh")
                    nc.gpsimd.tensor_add(mh2[:], a1n[:], a2n[:])
                    # emit next step's a2 matmuls NOW (decoupled from a1)
                    if s == GS - 1:
                        prz_nx = refs_next["przE"]
                        cs_nx = bass.ts(0, BL)
                    else:
                        prz_nx = refs["przE"] if (s + 1) % 2 == 0 else refs["przO"]
                        cs_nx = bass.ts((s + 1) // 2, BL)
                    pghn_nx = pghnp.tile([64, BL], F32, tag="pghn")
                    nc.tensor.matmul(
                        prz_nx[:, cs_nx], wrz, a2n[:],
                        start=False, stop=False, skip_group_check=True,
                    )
                    nc.tensor.matmul(pghn_nx[:], wn, a2n[:], start=True,
                                     stop=False, skip_group_check=True)
                    state["a1"], state["mh"] = a1n, mh2
                    state["pghn"] = pghn_nx

        def head(g, refs):
            vg, zg, xT = refs["vg"], refs["zg"], refs["xT"]
            p1 = pmisc.tile([128, COLS], F32, tag="pm")
            chunked(lambda sl: nc.tensor.matmul(
                p1[:, sl], lhsT1h[:], vg[:, sl], start=True, stop=False,
                skip_group_check=True))
            chunked(lambda sl: nc.tensor.matmul(
                p1[:, sl], lhsT1h[:], zg[:, sl], start=False, stop=False,
                skip_group_check=True))
            chunked(lambda sl: nc.tensor.matmul(
                p1[:, sl], lhsT1x[:], xT[:, sl], start=False, stop=True,
                skip_group_check=True))
            t1 = tmlp.tile([128, COLS], F32R, tag="t1")
            chunked(lambda sl: nc.scalar.activation(
                t1[:, sl], p1[:, sl], AF.Tanh, bias=bias1[:]))
            p2 = pmisc.tile([128, COLS], F32, tag="pm")
            chunked(lambda sl: nc.tensor.matmul(
                p2[:, sl], lhsT2[:], t1[:, sl], start=True, stop=True,
                skip_group_check=True))
            t2 = tmlp.tile([128, COLS], F32R, tag="t2")
            chunked(lambda sl: nc.scalar.activation(
                t2[:, sl], p2[:, sl], AF.Tanh, bias=bias2[:]))
            p3 = pmisc.tile([128, COLS], F32, tag="pm")
            chunked(lambda sl: nc.tensor.matmul(
                p3[:A + 1, sl], lhsT3[:], t2[:, sl],
                start=True, stop=True, skip_group_check=True))
            o7 = tmlp.tile([A + 1, COLS], F32, tag="o7")
            chunked(lambda sl: nc.vector.tensor_scalar_add(
                o7[:, sl], p3[:A + 1, sl], bias3[:]))

            po = pmisc.tile([128, GS // 2, A + 1], F32, tag="pm")
            for k in range(GS // 2):
                nc.tensor.transpose(
                    po[:, k, :], o7[:, k * 128:(k + 1) * 128], ident[:A + 1, :A + 1]
                )
            on = onp.tile([128, GS // 2, A + 1], F32, tag="on")
            nc.vector.tensor_copy(on[:], po[:])
            nc.sync.dma_start(
                out_d[g * GS:(g + 1) * GS].rearrange("(k ph) b j -> (ph b) k j", ph=2),
                on[:],
            )

        refs = bulk(0)
        # initial state: a1 = mb0 * h0T, a2 = 0, mh = a1
        a1_0 = small.tile([BL, BL], F32R, tag="a1")
        nc.vector.tensor_mul(a1_0[:], h0T[:], refs["mb"][:, 0:BL])
        a2_0 = small.tile([BL, BL], F32R, tag="a2")
        nc.scalar.copy(a2_0[:], zscratch[0:BL, 0:BL])
        pghn_0 = pghnp.tile([64, BL], F32, tag="pghn")
        nc.tensor.matmul(
            refs["przE"][:, bass.ts(0, BL)], wrz, a2_0[:],
            start=False, stop=False, skip_group_check=True,
        )
        nc.tensor.matmul(pghn_0[:], wn, a2_0[:], start=True, stop=False,
                         skip_group_check=True)
        state["a1"], state["mh"] = a1_0, a1_0
        state["pghn"] = pghn_0

        # head weights prepped after bulk(0)/chain-init: they share the
        # pmisc PSUM pool with bulk and aren't needed until head(0) (~22us in)
        lhsT1h = wp.tile([64, 128], F32R, tag="lhsT1h")
        lhsT1x = wp.tile([64, 128], F32R, tag="lhsT1x")
        for src, c0 in ((aw1_d, 0), (cw1_d, 64)):
            tmp = ldp.tile([128, 128], F32, tag="wtmp")
            nc.sync.dma_start(tmp[:L, :H + OBS], src[:, :])
            pt = pmisc.tile([128, COLS], F32, tag="pm")
            nc.tensor.transpose(pt[:H, :L], tmp[:L, 0:H], ident[:L, :L])
            nc.tensor.transpose(pt[:OBS, 128:128 + L], tmp[:L, H:H + OBS], ident[:L, :L])
            nc.scalar.copy(lhsT1h[:, c0:c0 + L], pt[:H, :L])
            nc.scalar.copy(lhsT1x[:, c0:c0 + L], pt[:OBS, 128:128 + L])

        lhsT2 = wp.tile([128, 128], F32R, tag="lhsT2")
        nc.scalar.copy(lhsT2[:], zscratch[:])
        for src, o in ((aw2_d, 0), (cw2_d, 64)):
            tmp = ldp.tile([128, 128], F32, tag="wtmp")
            nc.sync.dma_start(tmp[:L, :L], src[:, :])
            pt = pmisc.tile([128, COLS], F32, tag="pm")
            nc.tensor.transpose(pt[:L, :L], tmp[:L, :L], ident[:L, :L])
            nc.scalar.copy(lhsT2[o:o + L, o:o + L], pt[:L, :L])

        lhsT3 = wp.tile([128, A + 1], F32R, tag="lhsT3")
        nc.scalar.copy(lhsT3[:], zscratch[:, 0:A + 1])
        tmp = ldp.tile([128, 128], F32, tag="wtmp")
        nc.sync.dma_start(tmp[:A, :L], aw3_d[:, :])
        pt = pmisc.tile([128, COLS], F32, tag="pm")
        nc.tensor.transpose(pt[:L, :A], tmp[:A, :L], ident[:A, :A])
        nc.scalar.copy(lhsT3[:L, :A], pt[:L, :A])
        tmp = ldp.tile([128, 128], F32, tag="wtmp")
        nc.sync.dma_start(tmp[:1, :L], cw3_d[:, :])
        pt = pmisc.tile([128, COLS], F32, tag="pm")
        nc.tensor.transpose(pt[:L, :1], tmp[:1, :L], ident[:1, :1])
        nc.scalar.copy(lhsT3[64:64 + L, A:A + 1], pt[:L, :1])

        for g in range(1, ng):
            refs_next = bulk(g)
            chain(g - 1, refs, refs_next)
            head(g - 1, refs)
            refs = refs_next
        chain(ng - 1, refs, None)
        head(ng - 1, refs)

    return nc


_BUILT = {}


def get_built(t_loc=T):
    if t_loc not in _BUILT:
        nc = bacc.Bacc(None, target_bir_lowering=False)
        build(nc, t_loc)
        nc.compile()
        _BUILT[t_loc] = nc
    return _BUILT[t_loc]


def shard_inputs(inputs, t_loc=T):
    x = np.ascontiguousarray(np.asarray(inputs["x"], np.float32)).reshape(t_loc, B, OBS)
    done = np.ascontiguousarray(np.asarray(inputs["done"], np.float32)).reshape(t_loc, B)
    h0 = np.ascontiguousarray(np.asarray(inputs["gru_state"], np.float32)).reshape(B, H)
    common = {
        k: np.ascontiguousarray(np.asarray(inputs[k], np.float32))
        for k in WEIGHT_KEYS
    }
    in_maps = []
    for c in range(N_CORES):
        sl = slice(c * BL, (c + 1) * BL)
        m = dict(common)
        m["x"] = np.ascontiguousarray(x[:, sl, :])
        m["done"] = np.ascontiguousarray(done[:, sl])
        m["h0"] = np.ascontiguousarray(h0[sl, :])
        in_maps.append(m)
    return in_maps


def assemble_output(per_core_outs, t_loc=T):
    outs = [np.asarray(o, np.float32).reshape(t_loc, BL, A + 1) for o in per_core_outs]
    full = np.stack(outs, axis=1).reshape(t_loc, B, A + 1)
    return np.ascontiguousarray(full.reshape(t_loc * B, A + 1))


def run_on_hw(inputs, t_loc=T, trace=False, **kw):
    from concourse.bass_utils import run_bass_kernel_spmd

    nc = get_built(t_loc)
    in_maps = shard_inputs(inputs, t_loc)
    res = run_bass_kernel_spmd(
        nc, in_maps, core_ids=list(range(N_CORES)), trace=trace, **kw
    )
    out = assemble_output([r["out"] for r in res.results], t_loc)
    return out, res


def kernel(**inputs):
    out, _ = run_on_hw(inputs)
    return out
